# revision 7
# baseline (speedup 1.0000x reference)
"""GCN block (4 layers, shared weights) on 8 Trainium2 NeuronCores.

Math (per layer, PyG GCNConv):
    x' = relu(D^-1/2 (A+I) D^-1/2 (x W) + b)
Factorized: y = dinv * (x @ W);  agg[v] = sum_{(u,v) in E} y[u] + y[v];
    x'[v] = relu(dinv[v] * agg[v] + b)
so the edge phase needs no per-edge scaling.

Sharding: nodes split contiguously across 8 cores (12500 real + 44 pad rows
each). Each layer: local matmul (bf16 on the PE) -> AllGather of the y shard,
split into 4 partition-slices so later slices overlap the edge phase ->
dma_gather of edge sources from the replicated y -> aggregation on the
TensorEngine: edges are grouped by destination tile, each 128-edge slice is
multiplied by a one-hot selector matrix S (S[slot, m] = dst_mod128[slot]==m,
generated on the fly from an iota constant) and accumulated in PSUM, so no
scatter DMA is needed at all. Selector generation alternates between the
Vector and GpSimd engines to balance load.

Host-side preprocessing is index-only: bucketing by (target core, source
window, destination tile) plus the degree histogram (np.bincount of the
target ids); all floating-point math stays on the device.

Layouts: node-major [n, 64] DRAM regions use "partition-major" row order
r = (n % 128) * 98 + n // 128 so bulk SBUF<->DRAM transfers move 25KB per
partition contiguously while the gather addresses individual 256B rows.
"""

import numpy as np

N = 100000
F = 64
NC = 8
NLOC = 12500          # real nodes per core
T = 98                # 128-row tiles per core
NP = T * 128          # padded nodes per core = 12544
NSLICE_CC = 2         # how many AllGathers the y exchange is split into
PSL = 128 // NSLICE_CC
SLICE_ROWS = NC * PSL * T          # rows per y_full slice buffer
NWIN = 4                           # gather windows (int16 limit: 25088 rows)
WIN = NC * 32 * T                  # rows per gather window = 25088
WPS = NWIN // NSLICE_CC            # windows per cc slice
KMAX = 4096           # max gather slots per instruction (HW limit)
DEPTH = 4
ABLATE = set()        # timeline ablation: subset of {'mm','cc','edges','epi','tr'}


def _wrap16(idx, k, pad):
    """[k] int16 -> [128, k//16] wrapped in 16 partitions, replicated x8."""
    padded = np.full(k, pad, np.int16)
    padded[: len(idx)] = idx
    blk = padded.reshape(k // 16, 16).T
    return np.tile(blk, (8, 1))


def _build_schedule(edge_index):
    """Bucket edges by (target core, source window, dst tile) and pack into
    gather chunks of whole 128-slot slices.

    Returns (chunks, gi, dp) where chunks is a list of
    (k, window, [(t, nslices), ...]) shared by all cores, gi[c][i] the int16
    gather blob and dp[c][i] the fp32 dst-mod-128 blob for core c, chunk i.
    """
    src = np.asarray(edge_index[0], np.int64)
    dst = np.asarray(edge_index[1], np.int64)

    sc = src // NLOC
    sl = src - sc * NLOC
    sp, st = sl % 128, sl // 128
    scc = sp // PSL
    srow = sc * (PSL * T) + (sp % PSL) * T + st
    woff = srow // WIN
    swin = scc * WPS + woff
    srel = srow - woff * WIN

    dc = dst // NLOC
    dl = dst - dc * NLOC
    dt_, dp_ = dl // 128, dl % 128          # dst tile, dst row within tile

    # bucket[c][s][t] = (srel array, dp array)
    cnt = np.zeros((NC, NWIN, T), np.int64)
    buckets = [[[None] * T for _ in range(NWIN)] for _ in range(NC)]
    key = (dc * NWIN + swin) * T + dt_
    order = np.argsort(key, kind="stable")
    ks = key[order]
    bs_all, bp_all = srel[order], dp_[order]
    uniq, start = np.unique(ks, return_index=True)
    start = np.append(start, len(ks))
    for i, kk in enumerate(uniq):
        c, r = divmod(int(kk), NWIN * T)
        s, t = divmod(r, T)
        a, b = start[i], start[i + 1]
        buckets[c][s][t] = (bs_all[a:b], bp_all[a:b])
        cnt[c, s, t] = b - a

    gmax = ((cnt.max(axis=0) + 127) // 128) * 128     # [NWIN, T] group slots

    chunks = []          # (k, window, [(t, nsl), ...])
    for s in range(NWIN):
        cur, cur_slots = [], 0
        for t in range(T):
            g = int(gmax[s, t])
            if g == 0:
                continue
            if cur_slots + g > KMAX:
                chunks.append((cur_slots, s, cur))
                cur, cur_slots = [], 0
            cur.append((t, g // 128))
            cur_slots += g
        if cur:
            chunks.append((cur_slots, s, cur))

    gi = [[] for _ in range(NC)]
    dp = [[] for _ in range(NC)]
    for c in range(NC):
        for k, s, groups in chunks:
            gsl = np.zeros(k, np.int16)
            dsl = np.full(k, -1.0, np.float32)
            pos = 0
            for t, nsl in groups:
                g = nsl * 128
                bkt = buckets[c][s][t]
                if bkt is not None:
                    n = len(bkt[0])
                    gsl[pos:pos + n] = bkt[0]
                    dsl[pos:pos + n] = bkt[1]
                pos += g
            gi[c].append(_wrap16(gsl, k, 0))
            # dp layout matches gather output: slot i -> [i%128, i//128]
            dp[c].append(np.ascontiguousarray(
                dsl.reshape(k // 128, 128).T))
    return chunks, gi, dp


def _build_program(chunks):
    from concourse import bacc, tile
    from concourse import mybir

    f32, i16, bf16 = mybir.dt.float32, mybir.dt.int16, mybir.dt.bfloat16
    nc = bacc.Bacc("TRN2", target_bir_lowering=False, debug=False,
                   num_devices=NC, num_swdge_queues=2)

    NCH = len(chunks)
    xt_in = nc.dram_tensor("xt", [64, NP], bf16, kind="ExternalInput")
    w_in = nc.dram_tensor("W", [F, F], f32, kind="ExternalInput")
    b_in = nc.dram_tensor("b", [128, F], f32, kind="ExternalInput")
    id_in = nc.dram_tensor("ident", [128, 128], f32, kind="ExternalInput")
    iota_in = nc.dram_tensor("iota", [128, 128], f32, kind="ExternalInput")
    deg_in = nc.dram_tensor("deg", [128, T * F], f32, kind="ExternalInput")
    import os
    skip_edge_inputs = ('edges' in ABLATE
                        and os.environ.get("KERNEL_SKIP_EDGE_INPUTS"))
    if skip_edge_inputs:
        gi_in = dp_in = []
    else:
        gi_in = [nc.dram_tensor(f"gi{i}", [128, k // 16], i16,
                                kind="ExternalInput")
                 for i, (k, _, _) in enumerate(chunks)]
        dp_in = [nc.dram_tensor(f"dp{i}", [128, k // 128], f32,
                                kind="ExternalInput")
                 for i, (k, _, _) in enumerate(chunks)]
    out_d = nc.dram_tensor("out", [128, T * F], f32, kind="ExternalOutput")

    skip_cc_alloc = ('cc' in ABLATE and os.environ.get("KERNEL_SKIP_CC_ALLOC"))
    if skip_cc_alloc:
        y_loc, y_full = [], []
    else:
        y_loc = [nc.dram_tensor(f"y_loc{s}", [PSL, T * F], f32)
                 for s in range(NSLICE_CC)]
        y_full = [[nc.dram_tensor(f"y_full{i}_{s}", [SLICE_ROWS, F], f32,
                                  addr_space="Shared") for s in range(NSLICE_CC)]
                  for i in range(2)]

    add = mybir.AluOpType.add
    mult = mybir.AluOpType.mult
    iseq = mybir.AluOpType.is_equal
    Sqrt = mybir.ActivationFunctionType.Sqrt
    Copy = mybir.ActivationFunctionType.Copy
    Relu = mybir.ActivationFunctionType.Relu

    with tile.TileContext(nc) as tc:
        with tc.tile_pool(name="persist", bufs=1) as pp, \
             tc.tile_pool(name="msg", bufs=3) as mp, \
             tc.tile_pool(name="sel", bufs=6) as sp_, \
             tc.tile_pool(name="idx", bufs=6) as ip, \
             tc.tile_pool(name="ps", bufs=2, space="PSUM") as qp, \
             tc.tile_pool(name="psagg", bufs=3, space="PSUM") as qa:

            xT = pp.tile([64, NP], bf16)
            y_all = pp.tile([128, T * F], f32)
            aggws = pp.tile([128, T * F], f32)
            dinvb = pp.tile([128, T * F], f32)
            w_sb = pp.tile([F, F], bf16)
            w_f32 = pp.tile([F, F], f32)
            b_sb = pp.tile([128, F], f32)
            id_sb = pp.tile([128, 128], f32)
            iota_sb = pp.tile([128, 128], f32)

            nc.sync.dma_start(w_f32[:], w_in[:])
            nc.vector.tensor_copy(w_sb[:], w_f32[:])
            nc.sync.dma_start(b_sb[:], b_in[:])
            nc.sync.dma_start(id_sb[:], id_in[:])
            nc.sync.dma_start(iota_sb[:], iota_in[:])

            # dinv = 1/sqrt(deg), deg host-precomputed (index histogram)
            nc.sync.dma_start(dinvb[:], deg_in[:])
            nc.scalar.activation(dinvb[:], dinvb[:], Sqrt)
            nc.vector.reciprocal(dinvb[:], dinvb[:])

            # x -> xT: host-side transposed bf16 input
            nc.sync.dma_start(xT[:], xt_in[:])

            for l in range(DEPTH):
                yf = y_full[l % 2] if y_full else None
                # y = dinv * (x @ W)
                for t in range(T if 'mm' not in ABLATE else 0):
                    h = qp.tile([128, F], f32, tag="h")
                    nc.tensor.matmul(h[:], xT[:, t * 128:(t + 1) * 128],
                                     w_sb[:], start=True, stop=True)
                    nc.scalar.activation(
                        y_all[:, t * F:(t + 1) * F], h[:], Copy,
                        scale=dinvb[:, t * F: t * F + 1])
                # windowed allgather: slice s feeds its windows' chunks
                for s in range(NSLICE_CC if not skip_cc_alloc else 0):
                    nc.scalar.dma_start(y_loc[s][:],
                                        y_all[s * PSL:(s + 1) * PSL, :])
                    if 'cc' not in ABLATE:
                        nc.gpsimd.collective_compute(
                            "AllGather", mybir.AluOpType.bypass,
                            replica_groups=[list(range(NC))],
                            ins=[y_loc[s][:]], outs=[yf[s][:]])
                # edge phase: gather 256B rows, PE one-hot aggregation
                nc.vector.memset(aggws[:], 0.0)
                engs = [nc.vector, nc.gpsimd]
                ei = 0
                for ci in range(NCH if 'edges' not in ABLATE else 0):
                    k, s, groups = chunks[ci]
                    git = ip.tile([128, KMAX // 16], i16, tag="gi")
                    nc.sync.dma_start(git[:, : k // 16], gi_in[ci][:])
                    dpt = ip.tile([128, KMAX // 128], f32, tag="dp")
                    nc.sync.dma_start(dpt[:, : k // 128], dp_in[ci][:])
                    msg = mp.tile([128, KMAX // 128, F], f32, tag="msg")
                    nc.gpsimd.dma_gather(
                        msg[:, : k // 128, :],
                        yf[s // WPS][(s % WPS) * WIN:(s % WPS + 1) * WIN, :],
                        git[:, : k // 16], k, k, F, single_packet=False,
                        queue_num=0)
                    j = 0
                    for t, nsl in groups:
                        h = qa.tile([128, F], f32, tag="agg")
                        for u in range(nsl):
                            sel = sp_.tile([128, 128], f32, tag="sel")
                            engs[ei % 2].tensor_scalar(
                                sel[:], iota_sb[:], dpt[:, j:j + 1], None,
                                iseq)
                            ei += 1
                            nc.tensor.matmul(h[:], sel[:], msg[:, j, :],
                                             start=(u == 0),
                                             stop=(u == nsl - 1))
                            j += 1
                        nc.vector.tensor_tensor(
                            aggws[:, t * F:(t + 1) * F],
                            aggws[:, t * F:(t + 1) * F], h[:], add)
                # epilogue: x' = relu(dinv*(agg + y_self) + b)
                if 'epi' not in ABLATE:
                    nc.vector.tensor_tensor(aggws[:], aggws[:], y_all[:], add)
                    nc.vector.tensor_tensor(aggws[:], aggws[:], dinvb[:], mult)
                    for t in range(T):
                        nc.vector.tensor_tensor(
                            aggws[:, t * F:(t + 1) * F],
                            aggws[:, t * F:(t + 1) * F], b_sb[:], add)
                    nc.scalar.activation(aggws[:], aggws[:], Relu)
                if l < DEPTH - 1:
                    for t in range(T if 'tr' not in ABLATE else 0):
                        tr = qp.tile([64, 128], f32, tag="tr")
                        nc.tensor.transpose(
                            tr[:], aggws[:, t * F:(t + 1) * F], id_sb[:])
                        nc.vector.tensor_copy(
                            xT[:, t * 128:(t + 1) * 128], tr[:])
                else:
                    nc.scalar.dma_start(out_d[:], aggws[:])

            extra = int(os.environ.get("KERNEL_EXTRA_OPS", "0"))
            if extra:
                dummy = pp.tile([128, 64], f32)
                nc.vector.memset(dummy[:], 0.0)
                for _ in range(extra):
                    nc.vector.tensor_tensor(dummy[:], dummy[:], dummy[:],
                                            mult)

    nc.compile()
    return nc


def _host_inputs(x, W, b, edge_index):
    """Build the per-core in_maps (shared by kernel() and the bench)."""
    chunks, gi, dp = _build_schedule(edge_index)
    deg_full = np.bincount(np.asarray(edge_index[1], np.int64),
                           minlength=N).astype(np.float32) + 1.0
    b_bc = np.tile(np.asarray(b, np.float32)[None, :], (128, 1))
    ident = np.eye(128, dtype=np.float32)
    iota = np.tile(np.arange(128, dtype=np.float32)[None, :], (128, 1))
    in_maps = []
    import ml_dtypes
    for c in range(NC):
        xp = np.zeros((NP, F), np.float32)
        xp[:NLOC] = np.asarray(x, np.float32)[c * NLOC:(c + 1) * NLOC]
        xt = xp.reshape(T, 128, F).transpose(2, 0, 1).reshape(F, NP)
        dg = np.ones(NP, np.float32)
        dg[:NLOC] = deg_full[c * NLOC:(c + 1) * NLOC]
        deg_pm = np.repeat(
            dg.reshape(T, 128).T[:, :, None], F, axis=2).reshape(128, T * F)
        m = {"xt": np.ascontiguousarray(xt).astype(ml_dtypes.bfloat16),
             "W": np.asarray(W, np.float32), "b": b_bc, "ident": ident,
             "iota": iota, "deg": np.ascontiguousarray(deg_pm)}
        for i in range(len(chunks)):
            m[f"gi{i}"] = gi[c][i]
            m[f"dp{i}"] = dp[c][i]
        in_maps.append(m)
    return chunks, in_maps


last_results = None


def kernel(x, edge_index, batch_index, node_rankings, W, b, **_unused):
    import os
    from concourse.bass_utils import run_bass_kernel_spmd

    global last_results
    chunks, in_maps = _host_inputs(x, W, b, np.asarray(edge_index))
    nc = _build_program(chunks)
    res = run_bass_kernel_spmd(nc, in_maps, list(range(NC)),
                               trace=bool(os.environ.get("KERNEL_TRACE")))
    last_results = res

    out = np.empty((N, F), np.float32)
    for c in range(NC):
        o = res.results[c]["out"].reshape(128, T, F).transpose(1, 0, 2)
        out[c * NLOC:(c + 1) * NLOC] = o.reshape(NP, F)[:NLOC]
    return out

